# revision 6
# baseline (speedup 1.0000x reference)
"""Trainium2 Bass kernel for PersonaEmotionXModel (ragged span mean-pool +
persona attention + tiny MLP classifier).

Sharding: data-parallel over spans, split by span parity so each core needs
only one persona table. Cores 0-3 take even spans (pp_embedding), cores 4-7
take odd spans (yp_embedding); 1024 spans per core. Host gathers the
per-core [1024, 5] logits back into the full [8192, 5] output.
"""

import sys

sys.path.insert(0, "/opt/trn_rl_repo")

import numpy as np

import concourse.bacc as bacc
import concourse.bass as bass
import concourse.mybir as mybir
import concourse.tile as tile
from concourse.bass_utils import run_bass_kernel_spmd

FP = mybir.dt.float32
I32 = mybir.dt.int32
Alu = mybir.AluOpType
Act = mybir.ActivationFunctionType

S, N, SP, D = 65536, 8192, 512, 768
NCORES = 8
NS = N // NCORES  # spans per core = 1024
NG = NS // 128  # span groups of 128 = 8
NQ = NS * 8 // 128  # 128-token chunks per core = 64
H1 = 384
SELU_S = 1.0507009873554805
SELU_A = 1.6732632423543772
SA = SELU_S * SELU_A

_CACHE = {}


def _build_program():
    nc = bacc.Bacc(None, target_bir_lowering=False, debug=False)

    emb_d = nc.dram_tensor("emb", (NS * 8, D), FP, kind="ExternalInput")
    sep_d = nc.dram_tensor("sep", (128, NQ, 2), I32, kind="ExternalInput")
    per_d = nc.dram_tensor("persona", (SP, D), FP, kind="ExternalInput")
    w1_d = nc.dram_tensor("w1", (2 * D, H1), FP, kind="ExternalInput")
    b1_d = nc.dram_tensor("b1", (H1,), FP, kind="ExternalInput")
    w2_d = nc.dram_tensor("w2", (H1, 5), FP, kind="ExternalInput")
    b2_d = nc.dram_tensor("b2", (5,), FP, kind="ExternalInput")
    out_d = nc.dram_tensor("out", (NS, 5), FP, kind="ExternalOutput")

    with tile.TileContext(nc) as tc:
        with (
            tc.tile_pool(name="const", bufs=1) as cpool,
            tc.tile_pool(name="etile", bufs=12) as epool,
            tc.tile_pool(name="work", bufs=2) as wpool,
            tc.tile_pool(name="keep", bufs=1) as kpool,
            tc.tile_pool(name="ps_pt", bufs=2, space=bass.MemorySpace.PSUM) as pspt,
            tc.tile_pool(name="ps_big", bufs=2, space=bass.MemorySpace.PSUM) as psbig,
            tc.tile_pool(name="ps_sm", bufs=2, space=bass.MemorySpace.PSUM) as pssm,
        ):
            # ---------------- prep: constants and selector weights ----------
            # sep arrives token-major: sep[p, q, k] = sep_positions of the span
            # owning token p of 128-token chunk q (k=0 start, k=1 end, global).
            sep_t = cpool.tile([128, NQ, 2], I32)
            nc.sync.dma_start(sep_t[:], sep_d[:])

            lens_i = cpool.tile([128, NQ], I32)
            nc.vector.tensor_tensor(
                lens_i[:], sep_t[:, :, 1], sep_t[:, :, 0], Alu.subtract
            )
            lens_f = cpool.tile([128, NQ], FP)
            nc.vector.tensor_copy(lens_f[:], lens_i[:])
            recip_t = cpool.tile([128, NQ], FP)
            nc.vector.reciprocal(recip_t[:], lens_f[:])

            # slot index j = p & 7 per token partition
            pidx = cpool.tile([128, 1], I32)
            nc.gpsimd.iota(pidx[:], pattern=[[0, 1]], base=0, channel_multiplier=1)
            slot_i = cpool.tile([128, 1], I32)
            nc.vector.tensor_scalar(slot_i[:], pidx[:], 7, None, Alu.bitwise_and)
            slot_f = cpool.tile([128, 1], FP)
            nc.vector.tensor_copy(slot_f[:], slot_i[:])

            # wgt_tok[p, q] = (slot(p) < len) / len  for the span owning token p
            valid_t = cpool.tile([128, NQ], FP)
            nc.vector.tensor_scalar(valid_t[:], lens_f[:], slot_f[:], None, Alu.is_gt)
            wgt_tok = cpool.tile([128, NQ], FP)
            nc.vector.tensor_tensor(wgt_tok[:], valid_t[:], recip_t[:], Alu.mult)

            # diag16[p, a] = 1 iff a == p//8   (0 <= p - 8a <= 7)
            dval_i = cpool.tile([128, 16], I32)
            nc.gpsimd.iota(dval_i[:], pattern=[[-8, 16]], base=0, channel_multiplier=1)
            dval_f = cpool.tile([128, 16], FP)
            nc.vector.tensor_copy(dval_f[:], dval_i[:])
            m1 = cpool.tile([128, 16], FP)
            nc.vector.tensor_scalar(m1[:], dval_f[:], 0.0, None, Alu.is_ge)
            m2 = cpool.tile([128, 16], FP)
            nc.vector.tensor_scalar(m2[:], dval_f[:], 7.0, None, Alu.is_le)
            diag16 = cpool.tile([128, 16], FP)
            nc.vector.tensor_tensor(diag16[:], m1[:], m2[:], Alu.mult)

            # per-chunk selector: wsel[p, q, a] = diag16[p, a] * wgt_tok[p, q]
            wsel = cpool.tile([128, NQ, 16], FP)
            for q in range(NQ):
                nc.vector.tensor_scalar(
                    wsel[:, q, :], diag16[:], wgt_tok[:, q : q + 1], None, Alu.mult
                )

            # identity matrix for PE transposes
            ival_i = cpool.tile([128, 128], I32)
            nc.gpsimd.iota(ival_i[:], pattern=[[-1, 128]], base=0, channel_multiplier=1)
            ival_f = cpool.tile([128, 128], FP)
            nc.vector.tensor_copy(ival_f[:], ival_i[:])
            ident = cpool.tile([128, 128], FP)
            nc.vector.tensor_scalar(ident[:], ival_f[:], 0.0, None, Alu.is_equal)

            # classifier weights
            w1_sb = []
            for kc in range(12):
                t = cpool.tile([128, H1], FP, tag=f"w1sb{kc}", name=f"w1sb{kc}")
                nc.sync.dma_start(t[:], w1_d[kc * 128 : (kc + 1) * 128, :])
                w1_sb.append(t)
            w2_sb = []
            for mc in range(3):
                t = cpool.tile([128, 5], FP, tag=f"w2sb{mc}", name=f"w2sb{mc}")
                nc.sync.dma_start(t[:], w2_d[mc * 128 : (mc + 1) * 128, :])
                w2_sb.append(t)
            b1_sb = cpool.tile([128, 3], FP)
            nc.sync.dma_start(b1_sb[:], b1_d[:].rearrange("(mc p) -> p mc", p=128))
            sb1_sb = cpool.tile([128, 3], FP)  # SELU_S * b1
            nc.vector.tensor_scalar(sb1_sb[:], b1_sb[:], SELU_S, None, Alu.mult)
            b2_sb = cpool.tile([1, 5], FP)
            nc.sync.dma_start(b2_sb[:], b2_d[:].rearrange("(p c) -> p c", p=1))
            ones1 = cpool.tile([1, 128], FP)
            nc.vector.memset(ones1[:], 1.0)

            # persona, natural layout [512, 768] in 4 row-chunks
            p_nat = []
            for pc in range(4):
                t = cpool.tile([128, D], FP, tag=f"pnat{pc}", name=f"pnat{pc}")
                nc.sync.dma_start(t[:], per_d[pc * 128 : (pc + 1) * 128, :])
                p_nat.append(t)
            # personaT [768, 512] in 6 row-chunks of [128, 512]
            ppT = []
            for dc in range(6):
                t = cpool.tile([128, SP], FP, tag=f"ppT{dc}", name=f"ppT{dc}")
                ppT.append(t)
            for pc in range(4):
                for dc in range(6):
                    pst = pssm.tile([128, 128], FP, tag="sm", name="tpp", padded_shape=[128, 512])
                    nc.tensor.transpose(
                        pst[:], p_nat[pc][:, dc * 128 : (dc + 1) * 128], ident[:]
                    )
                    nc.scalar.copy(ppT[dc][:, pc * 128 : (pc + 1) * 128], pst[:])

            # ---------------- main loop: 2 batches of 4 span-groups ---------
            for bat in range(2):
                poolT = []  # 6 chunks of [128 d, 512 s] (4 groups wide)
                attT = []  # 6 chunks of [128 d, 512 s]
                for dc in range(6):
                    poolT.append(kpool.tile([128, 512], FP, tag=f"poolT{dc}", name=f"poolT{dc}"))
                    attT.append(kpool.tile([128, 512], FP, tag=f"attT{dc}", name=f"attT{dc}"))
                probsT = [wpool.tile([128, 512], FP, tag=f"probsT{pc}", name=f"probsT{pc}") for pc in range(4)]

                for gg in range(4):
                    g = bat * 4 + gg
                    # --- ragged mean-pool -> pooledT (6 psum chunks) ---
                    pt = pspt.tile([128, 6, 128], FP, tag="pt", name="pt")
                    for c in range(8):
                        q = g * 8 + c
                        et = epool.tile([128, D], FP, tag="et")
                        r0 = q * 128
                        nc.sync.dma_start(et[:], emb_d[r0 : r0 + 128, :])
                        for dsub in range(6):
                            nc.tensor.matmul(
                                pt[:, dsub, c * 16 : (c + 1) * 16],
                                et[:, dsub * 128 : (dsub + 1) * 128],
                                wsel[:, q, :],
                                start=True,
                                stop=True,
                            )
                    for dsub in range(6):
                        nc.scalar.copy(
                            poolT[dsub][:, gg * 128 : (gg + 1) * 128], pt[:, dsub, :]
                        )

                    # --- attention scores + softmax ---
                    ps_s = psbig.tile([128, SP], FP, tag="big", name="ps_s")
                    for dc in range(6):
                        nc.tensor.matmul(
                            ps_s[:],
                            poolT[dc][:, gg * 128 : (gg + 1) * 128],
                            ppT[dc][:],
                            start=(dc == 0),
                            stop=(dc == 5),
                        )
                    mx = wpool.tile([128, 1], FP, tag="mx")
                    nc.vector.tensor_reduce(mx[:], ps_s[:], mybir.AxisListType.X, Alu.max)
                    nmx = wpool.tile([128, 1], FP, tag="nmx")
                    nc.vector.tensor_scalar(nmx[:], mx[:], -1.0, None, Alu.mult)
                    probs = wpool.tile([128, SP], FP, tag="probs")
                    rsum = wpool.tile([128, 1], FP, tag="rsum")
                    nc.scalar.activation(
                        probs[:], ps_s[:], Act.Exp, bias=nmx[:], accum_out=rsum[:]
                    )
                    rinv = wpool.tile([128, 1], FP, tag="rinv")
                    nc.vector.reciprocal(rinv[:], rsum[:])
                    pnorm = wpool.tile([128, SP], FP, tag="pnorm")
                    nc.vector.tensor_scalar(pnorm[:], probs[:], rinv[:], None, Alu.mult)
                    for pc in range(4):
                        tps = pssm.tile([128, 128], FP, tag="sm", name="tprob", padded_shape=[128, 512])
                        nc.tensor.transpose(
                            tps[:], pnorm[:, pc * 128 : (pc + 1) * 128], ident[:]
                        )
                        nc.scalar.copy(probsT[pc][:, gg * 128 : (gg + 1) * 128], tps[:])

                # --- attended^T = persona^T @ probs^T  (batch of 4 groups) ---
                for dc in range(6):
                    ps_a = psbig.tile([128, 512], FP, tag="big", name="ps_a")
                    for pc in range(4):
                        nc.tensor.matmul(
                            ps_a[:],
                            p_nat[pc][:, dc * 128 : (dc + 1) * 128],
                            probsT[pc][:],
                            start=(pc == 0),
                            stop=(pc == 3),
                        )
                    nc.scalar.copy(attT[dc][:], ps_a[:])

                # --- MLP: hT = selu(W1^T @ finalT + b1) ---
                h_sb = []
                for mc in range(3):
                    ps_h = psbig.tile([128, 512], FP, tag="big", name="ps_h")
                    for kc in range(12):
                        rhs = poolT[kc] if kc < 6 else attT[kc - 6]
                        nc.tensor.matmul(
                            ps_h[:],
                            w1_sb[kc][:, mc * 128 : (mc + 1) * 128],
                            rhs[:],
                            start=(kc == 0),
                            stop=(kc == 11),
                        )
                    b1c = b1_sb[:, mc : mc + 1]
                    sb1c = sb1_sb[:, mc : mc + 1]
                    xm = wpool.tile([128, 512], FP, tag="xm")
                    nc.vector.tensor_scalar(xm[:], ps_h[:], b1c, 0.0, Alu.add, Alu.min)
                    rl = wpool.tile([128, 512], FP, tag="rl")
                    nc.scalar.activation(rl[:], ps_h[:], Act.Relu, bias=sb1c, scale=SELU_S)
                    ex = wpool.tile([128, 512], FP, tag="ex")
                    nc.scalar.activation(ex[:], xm[:], Act.Exp)
                    e2 = wpool.tile([128, 512], FP, tag="e2")
                    nc.vector.tensor_scalar(e2[:], ex[:], SA, -SA, Alu.mult, Alu.add)
                    ht = wpool.tile([128, 512], FP, tag=f"ht{mc}", name=f"ht{mc}")
                    nc.vector.tensor_tensor(ht[:], e2[:], rl[:], Alu.add)
                    h_sb.append(ht)

                # --- logits + softmax ---
                for gg in range(4):
                    g = bat * 4 + gg
                    ps_l = pssm.tile([128, 5], FP, tag="sm", name="ps_l", padded_shape=[128, 512])
                    for mc in range(3):
                        nc.tensor.matmul(
                            ps_l[:],
                            h_sb[mc][:, gg * 128 : (gg + 1) * 128],
                            w2_sb[mc][:],
                            start=(mc == 0),
                            stop=False,
                        )
                    nc.tensor.matmul(
                        ps_l[:], ones1[:], b2_sb[:], start=False, stop=True
                    )
                    lmx = wpool.tile([128, 1], FP, tag="lmx")
                    nc.vector.tensor_reduce(
                        lmx[:], ps_l[:], mybir.AxisListType.X, Alu.max
                    )
                    lnm = wpool.tile([128, 1], FP, tag="lnm")
                    nc.vector.tensor_scalar(lnm[:], lmx[:], -1.0, None, Alu.mult)
                    lex = wpool.tile([128, 5], FP, tag="lex")
                    lsum = wpool.tile([128, 1], FP, tag="lsum")
                    nc.scalar.activation(
                        lex[:], ps_l[:], Act.Exp, bias=lnm[:], accum_out=lsum[:]
                    )
                    linv = wpool.tile([128, 1], FP, tag="linv")
                    nc.vector.reciprocal(linv[:], lsum[:])
                    ot = wpool.tile([128, 5], FP, tag="ot")
                    nc.vector.tensor_scalar(ot[:], lex[:], linv[:], None, Alu.mult)
                    nc.sync.dma_start(out_d[g * 128 : (g + 1) * 128, :], ot[:])

    nc.compile()
    return nc


def _span_ids(core):
    parity = 0 if core < 4 else 1
    blk = core % 4
    return np.arange(blk * NS, (blk + 1) * NS) * 2 + parity


def kernel(embedding, pp_embedding, yp_embedding, W1, b1, W2, b2, sep_positions):
    if "nc" not in _CACHE:
        _CACHE["nc"] = _build_program()
    nc = _CACHE["nc"]

    emb3 = np.ascontiguousarray(embedding, dtype=np.float32).reshape(N, 8, D)
    sep = np.asarray(sep_positions).astype(np.int32)
    a_over_8 = (np.arange(128) // 8)[:, None]
    q_idx = np.arange(NQ)[None, :]

    in_maps = []
    for core in range(NCORES):
        ids = _span_ids(core)
        # token-major replicated sep: sep_tok[p, q, :] = sep[ids[16q + p//8], :]
        sep_core = sep[ids]  # [1024, 2]
        sep_tok = sep_core.reshape(NQ, 16, 2)[q_idx, a_over_8, :]
        in_maps.append(
            {
                "emb": np.ascontiguousarray(emb3[ids]).reshape(NS * 8, D),
                "sep": np.ascontiguousarray(sep_tok),
                "persona": np.asarray(
                    pp_embedding if core < 4 else yp_embedding, dtype=np.float32
                ),
                "w1": np.asarray(W1, dtype=np.float32),
                "b1": np.asarray(b1, dtype=np.float32),
                "w2": np.asarray(W2, dtype=np.float32),
                "b2": np.asarray(b2, dtype=np.float32),
            }
        )

    res = run_bass_kernel_spmd(nc, in_maps, core_ids=list(range(NCORES)))
    _CACHE["last_result"] = res

    out = np.empty((N, 5), dtype=np.float32)
    for core in range(NCORES):
        out[_span_ids(core)] = res.results[core]["out"]
    return out
